# revision 16
# baseline (speedup 1.0000x reference)
"""CANModule forward kernel for 8 Trainium2 NeuronCores.

The reference computes
    new_place = relu(place_cells + ec @ W_ec + sum_i grid_i @ W_mec_i)
(the MEC grid updates are computed-then-deleted in the reference - dead
code - so W_gh*/W_gg* never need to reach the device).

Strategy: shard the HPC output dim (8192) column-wise across 8 cores
(1024 cols each).  Weights are quantized host-side to fp8 E3M4 (x32 so
the sigma~1/64..1/32 gaussians land in e3m4's normal range), halving
HBM traffic vs fp16 - the kernel is memory-bound.  The 1/32 descale is
folded into A on the host.

Per core the matmul runs W-STATIONARY: for each 128-wide tile t of the
1024 output cols and each of 88 K-chunks,
    psum_t[128, 4] += W[k-chunk, t-tile][128, 128].T-as-lhsT @ A_k[128, 4]
so W is ingested through LDWEIGHTS with FastWeightLoad (4 fp8/cycle)
instead of streaming as the moving operand (1 elem/cycle).  That keeps
PE time (~28us) under the fp8 DMA floor (~33us).  A small warmup burst
of dummy matmuls un-throttles the PE clock (HAM) before real data lands.

The place bias is applied for free in the final ACT relu via its bias
operand; out.T tiles [128, 4x8] are stored and re-assembled on host.
"""

import numpy as np
import ml_dtypes

import concourse.bass as bass
import concourse.mybir as mybir
import concourse.tile as tile
from concourse.bass_utils import run_bass_kernel_spmd

N_CORES = 8
B = 4
EC = 4096
MECS = (1024, 2048, 4096)
HPC = 8192
SHARD = HPC // N_CORES          # 1024 output cols per core
K_TOTAL = EC + sum(MECS)        # 11264 contraction rows
P = 128
KC = K_TOTAL // P               # 88 K-chunks
T = SHARD // P                  # 8 col-tiles of 128 output cols
W_SCALE = 32.0                  # e3m4 range fit; 1/32 folded into A

CONFIG = {
    "trace": False,
    "a_dtype": "f16",           # "f16" (mixed-dtype matmul) or "f8e3"
    "strip_ceremony": True,
    # W DMA pieces per col-tile, by tile: big early pieces maximize the
    # bytes queued ahead of the HWDGE ring-capacity backpressure; small late
    # pieces shrink the straggler-engine completion skew on the tail
    "pieces": (1, 1, 1, 1, 2, 2, 4, 4),
    "relu_dve": True,           # relu+bias on VectorE (frees the scalar ring)
}
_CACHE = {}


def _a_dts():
    if CONFIG["a_dtype"] == "f16":
        return mybir.dt.float16, np.float16
    return mybir.dt.float8e3, ml_dtypes.float8_e3m4


def _build():
    DT_A, _ = _a_dts()
    PIECES = CONFIG["pieces"]
    act_scale = 1.0 if CONFIG["a_dtype"] == "f16" else 1.0 / W_SCALE
    relu_dve = CONFIG["relu_dve"] and act_scale == 1.0

    nc = bass.Bass()
    a = nc.dram_tensor("a", [P, KC * B], DT_A, kind="ExternalInput")
    pl = nc.dram_tensor("pl", [P, T], mybir.dt.float32, kind="ExternalInput")
    w = nc.dram_tensor("w", [T * P, KC * P], mybir.dt.float8e3, kind="ExternalInput")
    out = nc.dram_tensor("out", [P, T * B], mybir.dt.float32, kind="ExternalOutput")

    with tile.TileContext(nc) as tc:
        with (
            tc.tile_pool(name="const", bufs=1) as cpool,
            tc.tile_pool(name="wload", bufs=1) as wpool,
            tc.tile_pool(name="outp", bufs=1) as opool,
            tc.tile_pool(name="acc", bufs=1, space="PSUM") as pspool,
        ):
            a_t = cpool.tile([P, KC * B], DT_A)
            pl_t = cpool.tile([P, T], mybir.dt.float32)
            warm_t = cpool.tile([P, P], DT_A)
            o_t = opool.tile([P, T * B], mybir.dt.float32)

            nc.vector.memset(warm_t[:], 0.0)

            ps_tiles = []
            for t in range(T):
                pst = pspool.tile([P, B + B], mybir.dt.float32, name=f"ps{t}")
                ps_tiles.append(pst)

            # W pieces first on both HWDGE rings; the tiny a/pl transfers ride
            # behind the first piece of each ring (they are needed no earlier)
            w_r = w.rearrange("(t p) m -> t p m", p=P)
            w_tiles = []
            n_dma = 0
            for t in range(T):
                nspl = PIECES[t]
                kc_cut = [KC * i // nspl for i in range(nspl + 1)]
                for i in range(nspl):
                    k0, k1 = kc_cut[i], kc_cut[i + 1]
                    wt = wpool.tile(
                        [P, (k1 - k0) * P], mybir.dt.float8e3, name=f"w{t}_{i}"
                    )
                    eng = nc.sync if n_dma % 2 == 0 else nc.scalar
                    eng.dma_start(wt[:], w_r[t][:, k0 * P : k1 * P])
                    w_tiles.append((t, wt, k0, k1))
                    n_dma += 1
                    if n_dma == 1:
                        nc.sync.dma_start(a_t[:], a[:])
                    elif n_dma == 2:
                        nc.scalar.dma_start(pl_t[:], pl[:])

            # gate: first A read on the PE stream; later matmuls then carry at
            # most one sem wait (their W-piece DMA)
            nc.tensor.matmul(
                ps_tiles[-1][:, B : 2 * B],
                warm_t[:, 0:P],
                a_t[:, 0:B],
                start=True,
                stop=True,
            )
            if relu_dve:
                # gate the pl load on the (in-order) DVE stream so each relu
                # below carries only its PE wait
                gate_t = opool.tile([P, 1], mybir.dt.float32)
                nc.vector.tensor_scalar(
                    gate_t[:],
                    pl_t[:, 0:1],
                    0.0,
                    None,
                    mybir.AluOpType.add,
                )

            for t, wt, k0, k1 in w_tiles:
                ps = ps_tiles[t][:, 0:B]
                for k in range(k0, k1):
                    c = (k - k0) * P
                    nc.tensor.matmul(
                        ps,
                        wt[:, c : c + P],
                        a_t[:, B * k : B * (k + 1)],
                        start=(k == 0),
                        stop=(k == KC - 1),
                    )
                if k1 == KC:
                    if relu_dve:
                        nc.vector.tensor_scalar(
                            o_t[:, B * t : B * (t + 1)],
                            ps,
                            pl_t[:, t : t + 1],
                            0.0,
                            mybir.AluOpType.add,
                            mybir.AluOpType.max,
                        )
                    else:
                        nc.scalar.activation(
                            o_t[:, B * t : B * (t + 1)],
                            ps,
                            mybir.ActivationFunctionType.Relu,
                            bias=pl_t[:, t : t + 1],
                            scale=act_scale,
                        )
            nc.sync.dma_start(out[:], o_t[:])

    _strip_redundant_waits(nc)
    if CONFIG["strip_ceremony"]:
        _strip_ceremony(nc)
    return nc


def _strip_redundant_waits(nc):
    """The DMA / Matmult / Drain pseudo-ops encode a single sync wait, but
    Tile can emit more.

    1. The output-store DMA gets {Activation >= 8, DMAHW_k >= 16}.  The
       Activation wait implies the DMA wait transitively: ACT is in-order and
       every ACT is gated on PE progress whose matmuls waited on that W DMA.
    2. The end-of-kernel quiesce drain waits on every proc lane, but the
       kernel is one dependency chain ending in the output-store DMA:
       "store landed" implies everything else.
    """
    insts = [i for blk in nc.m.functions[0].blocks for i in blk.instructions]
    for inst in insts:
        ty = type(inst).__name__
        si = inst.sync_info
        if si is None or len(si.on_wait) <= 1:
            continue
        if ty == "InstDMACopy":
            waits = list(si.on_wait)
            engine = [
                w
                for w in waits
                if w.ant_name.split("_")[0] in ("PE", "Activation", "DVE", "Pool", "SP")
            ]
            rest = [w for w in waits if w not in engine]
            dma_lanes = [w for w in rest if w.ant_name.startswith("DMA")]
            if len(engine) == 1 and len(dma_lanes) == len(rest):
                si.on_wait = engine
                continue
        if ty in ("InstDMACopy", "InstMatmult"):
            raise RuntimeError(
                f"{inst.name} ({ty}) still has {len(si.on_wait)} waits: {si}"
            )

    store = [i for i in insts if type(i).__name__ == "InstDMACopy"][-1]
    assert store.sync_info and len(store.sync_info.on_update) == 1
    lane = store.sync_info.on_update[0].ant_name
    cum = 0
    for i in insts:
        if i.sync_info:
            cum += sum(
                u.update_value for u in i.sync_info.on_update if u.ant_name == lane
            )
    for inst in insts:
        if type(inst).__name__ != "InstDrain":
            continue
        si = inst.sync_info
        if si is None or len(si.on_wait) <= 1:
            continue
        keep = [w for w in si.on_wait if w.ant_name == lane and w.wait_value == cum]
        assert keep, f"drain {inst.name} lacks the store-lane wait (cum={cum}): {si}"
        si.on_wait = keep[:1]


def _strip_ceremony(nc):
    """Remove the all-engine butterfly barriers that bracket the kernel.

    Every data dependency is carried by absolute-valued semaphore waits from
    a zeroed sem file, so engines may enter their streams unaligned.
    """
    blocks = nc.m.functions[0].blocks
    b0 = blocks[0]
    drop = [
        n
        for n, i in enumerate(b0.instructions)
        if type(i).__name__ in ("InstDrain", "InstEventSemaphore")
    ]
    for n in reversed(drop):
        del b0.instructions[n]

    end = blocks[-1]
    isa_idx = [
        n for n, i in enumerate(end.instructions) if type(i).__name__ == "InstISA"
    ]
    if isa_idx:
        for n in range(len(end.instructions) - 1, isa_idx[-1], -1):
            del end.instructions[n]


def kernel(**inputs):
    _, np_a = _a_dts()
    ec = np.asarray(inputs["ec_activations"], dtype=np.float32)
    place = np.asarray(inputs["place_cells"], dtype=np.float32)
    grids = [np.asarray(inputs[f"grid{i}"], dtype=np.float32) for i in range(3)]
    W_ec = np.asarray(inputs["W_ec"], dtype=np.float32)
    W_mec = [np.asarray(inputs[f"W_mec{i}"], dtype=np.float32) for i in range(3)]

    X = np.concatenate(grids, axis=1)                                   # [1, 7168]
    A = np.concatenate([ec, np.broadcast_to(X, (B, X.shape[1]))], 1)    # [4, 11264]
    if CONFIG["a_dtype"] == "f16":
        A = A / W_SCALE          # fold the W descale into A (else: ACT rescales)
    # swizzle A.T into [p, (k b)] chunk-major layout
    aT_sw = np.ascontiguousarray(
        A.T.reshape(KC, P, B).transpose(1, 0, 2)
    ).reshape(P, KC * B).astype(np_a)

    W_all = np.concatenate([W_ec] + W_mec, axis=0)                      # [11264, 8192]
    Wq = (W_all * W_SCALE).astype(ml_dtypes.float8_e3m4)

    key = "nc_" + CONFIG["a_dtype"] + str(CONFIG["strip_ceremony"])
    nc = _CACHE.get(key)
    if nc is None:
        nc = _CACHE[key] = _build()

    in_maps = []
    for c in range(N_CORES):
        cols = slice(SHARD * c, SHARD * (c + 1))
        # [t*128+p, k*128+j] = Wq[k*128+p, t*128+j]
        w_sw = np.ascontiguousarray(
            Wq[:, cols].reshape(KC, P, T, P).transpose(2, 1, 0, 3)
        ).reshape(T * P, KC * P)
        pl_sw = np.ascontiguousarray(
            place[0, cols].reshape(T, P).T
        ).astype(np.float32)
        in_maps.append({"a": aT_sw, "pl": pl_sw, "w": w_sw})
    res = run_bass_kernel_spmd(
        nc, in_maps, core_ids=list(range(N_CORES)), trace=CONFIG["trace"]
    )
    _CACHE["last_results"] = res
    outs = []
    for c in range(N_CORES):
        o = np.asarray(res.results[c]["out"])                           # [128, 8*4]
        outs.append(o.reshape(P, T, B).transpose(2, 1, 0).reshape(B, SHARD))
    return np.concatenate(outs, axis=1)
